# revision 8
# baseline (speedup 1.0000x reference)
"""Trainium2 Bass kernel for a 2-layer GCN encoder (AssemblyQueryEncoder).

Reference computation (PyG-style GCNConv x2 + global mean pool + linear + L2norm):
    h1 = relu(gcnconv(x, W1, b1));  h2 = relu(gcnconv(h1, W2, b2))
    g  = segment_mean(h2, batch) @ Wl + bl;  out = g / max(||g||_2, eps)

Distribution over 8 NeuronCores:
  - Nodes are sharded contiguously (5120 padded nodes/core); each core owns the
    incoming edges of its nodes (partitioned by destination).
  - Per layer: local transform h = x @ W, AllGather of h ([40960,128]) so every
    core can gather arbitrary source rows (dma_gather, int16 indices -> edges are
    split into a "lo" stream (src < 32768) and "hi" stream (src >= 32768) that
    gather from two views of the same table), then per-destination-block
    aggregation computed as selection-matrix matmuls accumulated in PSUM.
    Self-loop term is folded in as extra edges (i->i weight dinv2[i]); bias is
    folded in as a rank-1 (K=1) matmul.
  - Pooled per-graph sums (1/count folded into the pooling matrix) are
    AllReduced ([128,64]); final linear + L2 norm computed redundantly.
"""

import sys

sys.path.insert(0, "/opt/trn_rl_repo")

import numpy as np

P = 128  # partitions
LO = 32768  # int16-addressable rows


def _cdiv(a, b):
    return (a + b - 1) // b


class GCNConfig:
    def __init__(self, n_nodes=40000, n_graphs=64, d_in=128, d_hid=128, d_out=64,
                 n_cores=8, chunk_tiles=8):
        self.n_nodes = n_nodes
        self.n_graphs = n_graphs
        self.d_in = d_in
        self.d_hid = d_hid
        self.d_out = d_out
        self.n_cores = n_cores
        self.chunk_tiles = chunk_tiles
        self.nloc = _cdiv(n_nodes, n_cores * P) * P  # padded nodes per core
        self.npad = self.nloc * n_cores
        self.nblk = self.nloc // P  # 128-node blocks per core


def _wrap_idx(flat):
    """dma_gather index layout: element i -> [i % 16, i // 16], x8 partitions."""
    n = flat.shape[0]
    assert n % 16 == 0
    arr = np.zeros((16, n // 16), np.int16)
    arr[np.arange(n) % 16, np.arange(n) // 16] = flat
    return np.tile(arr, (8, 1))


def preprocess(cfg, x, edge_index, batch):
    """Host-side index preprocessing. Edges (incl. self-loops) are grouped per
    core by destination block, split into lo/hi streams by source id, each
    (block, stream) list padded to a tile multiple shared by all cores."""
    n, nc_ = cfg.n_nodes, cfg.n_cores
    lo_split = min(LO, cfg.npad)
    src = np.asarray(edge_index[0], dtype=np.int64)
    dst = np.asarray(edge_index[1], dtype=np.int64)
    batch = np.asarray(batch, dtype=np.int64)

    deg = np.bincount(dst, minlength=n).astype(np.float64) + 1.0
    dinv = 1.0 / np.sqrt(deg)
    norm = (dinv[src] * dinv[dst]).astype(np.float32)
    dinv2 = (dinv * dinv).astype(np.float32)

    loops = np.arange(n, dtype=np.int64)
    src_a = np.concatenate([src, loops])
    dst_a = np.concatenate([dst, loops])
    nrm_a = np.concatenate([norm, dinv2]).astype(np.float32)

    # sort by (stream, dst): stream 0 = lo sources, 1 = hi
    strm = (src_a >= lo_split).astype(np.int64)
    order = np.lexsort((dst_a, strm))
    src_a, dst_a, nrm_a, strm = src_a[order], dst_a[order], nrm_a[order], strm[order]
    n_lo = int((strm == 0).sum())

    nblk_g = cfg.npad // P
    res = {}
    for sname, lo_, hi_ in (("A", 0, n_lo), ("B", n_lo, len(src_a))):
        s_s = src_a[lo_:hi_]
        s_d = dst_a[lo_:hi_]
        s_n = nrm_a[lo_:hi_]
        blk = s_d // P
        counts = np.bincount(blk, minlength=nblk_g).reshape(nc_, cfg.nblk)
        T = _cdiv_arr(counts.max(axis=0), P).astype(np.int64)  # may be 0
        ttot = max(int(T.sum()), 1)
        tstart = np.concatenate([[0], np.cumsum(T)]).astype(np.int64)
        bstart = np.concatenate(
            [[0], np.cumsum(np.bincount(blk, minlength=nblk_g))]).astype(np.int64)
        gidx = np.zeros((nc_, P, ttot), np.int16)
        dstf = np.zeros((nc_, P, ttot), np.float32)
        nrmf = np.zeros((nc_, P, ttot), np.float32)
        for c in range(nc_):
            for b in range(cfg.nblk):
                gb = c * cfg.nblk + b
                e0, e1 = bstart[gb], bstart[gb + 1]
                m = e1 - e0
                if m == 0:
                    continue
                jj = np.arange(m)
                pp, tt = jj % P, tstart[b] + jj // P
                gidx[c, pp, tt] = (s_s[e0:e1]
                                   - (lo_split if sname == "B" else 0))
                dstf[c, pp, tt] = (s_d[e0:e1] % P).astype(np.float32)
                nrmf[c, pp, tt] = s_n[e0:e1]
        # wrapped int16 index array [128, ttot*8]
        widx = np.stack([_wrap_idx(gidx[c].T.reshape(-1)) for c in range(nc_)])
        res["gidx" + sname] = widx
        res["dstf" + sname] = dstf
        res["nrmf" + sname] = nrmf
        res["T" + sname] = T
        res["ttot" + sname] = ttot

    # x transposed per core, padded
    xT = np.zeros((nc_, cfg.d_in, cfg.nloc), np.float32)
    xf = np.asarray(x, dtype=np.float32)
    for c in range(nc_):
        lo2, hi2 = c * cfg.nloc, min((c + 1) * cfg.nloc, n)
        if hi2 > lo2:
            xT[c, :, : hi2 - lo2] = xf[lo2:hi2].T

    # pooling matrix with 1/count folded in, block-major [P, nblk*G]
    g_ = cfg.n_graphs
    cnt = np.maximum(np.bincount(batch, minlength=g_).astype(np.float32), 1.0)
    pm = np.zeros((nc_, P, cfg.nblk * g_), np.float32)
    for c in range(nc_):
        for b in range(cfg.nblk):
            base = c * cfg.nloc + b * P
            hi2 = min(base + P, n)
            if hi2 <= base:
                continue
            rows = np.arange(hi2 - base)
            gg = batch[base:hi2]
            pm[c, rows, b * g_ + gg] = 1.0 / cnt[gg]

    res.update(xT=xT, pm=pm)
    return res


def _cdiv_arr(a, b):
    return (a + b - 1) // b


def build(cfg, TA, ttotA, TB, ttotB):
    """Build the SPMD Bass graph (same program for all cores)."""
    import concourse.mybir as mybir
    import concourse.tile as tile
    from concourse import bacc
    from concourse.masks import make_identity

    f32 = mybir.dt.float32
    i16 = mybir.dt.int16
    AF = mybir.ActivationFunctionType
    ALU = mybir.AluOpType

    nc_ = cfg.n_cores
    nblk = cfg.nblk
    nloc = cfg.nloc
    npad = cfg.npad
    lo_split = min(LO, npad)
    dh = cfg.d_hid
    do = cfg.d_out
    g_ = cfg.n_graphs
    rg = [list(range(nc_))]
    chunk = cfg.chunk_tiles

    nc = bacc.Bacc("TRN2", target_bir_lowering=False, debug=False,
                   num_devices=nc_)

    # ---- parameters ----
    xT_p = nc.declare_dram_parameter("xT", [cfg.d_in, nloc], f32, isOutput=False)
    gidxA_p = nc.declare_dram_parameter("gidxA", [P, ttotA * 8], i16, isOutput=False)
    dstfA_p = nc.declare_dram_parameter("dstfA", [P, ttotA], f32, isOutput=False)
    nrmfA_p = nc.declare_dram_parameter("nrmfA", [P, ttotA], f32, isOutput=False)
    gidxB_p = nc.declare_dram_parameter("gidxB", [P, ttotB * 8], i16, isOutput=False)
    dstfB_p = nc.declare_dram_parameter("dstfB", [P, ttotB], f32, isOutput=False)
    nrmfB_p = nc.declare_dram_parameter("nrmfB", [P, ttotB], f32, isOutput=False)
    pm_p = nc.declare_dram_parameter("pm", [P, nblk * g_], f32, isOutput=False)
    w1_p = nc.declare_dram_parameter("W1", [cfg.d_in, dh], f32, isOutput=False)
    w2_p = nc.declare_dram_parameter("W2", [dh, dh], f32, isOutput=False)
    wl_p = nc.declare_dram_parameter("Wl", [dh, do], f32, isOutput=False)
    b1_p = nc.declare_dram_parameter("b1", [1, dh], f32, isOutput=False)
    b2_p = nc.declare_dram_parameter("b2", [1, dh], f32, isOutput=False)
    bl_p = nc.declare_dram_parameter("bl", [1, do], f32, isOutput=False)
    out_p = nc.declare_dram_parameter("out", [g_, do], f32, isOutput=True)

    # ---- internal DRAM ----
    agin1 = nc.dram_tensor("agin1", [nloc, dh], f32)
    table1 = nc.dram_tensor("table1", [npad, dh], f32, addr_space="Shared")
    agin2 = nc.dram_tensor("agin2", [nloc, dh], f32)
    table2 = nc.dram_tensor("table2", [npad, dh], f32, addr_space="Shared")
    arin = nc.dram_tensor("arin", [dh, g_], f32)
    arout = nc.dram_tensor("arout", [dh, g_], f32, addr_space="Shared")

    tstartA = np.concatenate([[0], np.cumsum(TA)]).astype(np.int64)
    tstartB = np.concatenate([[0], np.cumsum(TB)]).astype(np.int64)

    with tile.TileContext(nc) as tc:
        with (
            tc.tile_pool(name="const", bufs=1) as cpool,
            tc.tile_pool(name="big", bufs=1) as bigpool,
            tc.tile_pool(name="gat", bufs=4) as gpool,
            tc.tile_pool(name="m", bufs=6) as mpool,
            tc.tile_pool(name="small", bufs=2) as spool,
            tc.tile_pool(name="psum", bufs=3, space="PSUM") as pspool,
            tc.tile_pool(name="psum1", bufs=1, space="PSUM") as pspool1,
        ):
            # ---- constants ----
            w1_sb = cpool.tile([cfg.d_in, dh], f32)
            w2_sb = cpool.tile([dh, dh], f32)
            wl_sb = cpool.tile([dh, do], f32)
            b1_sb = cpool.tile([1, dh], f32)
            b2_sb = cpool.tile([1, dh], f32)
            bl_sb = cpool.tile([1, do], f32)
            ones_sb = cpool.tile([1, P], f32)
            ident_sb = cpool.tile([P, P], f32)
            iota_i = cpool.tile([P, P], mybir.dt.int32)
            iota_f = cpool.tile([P, P], f32)
            nc.sync.dma_start(w1_sb[:], w1_p[:])
            nc.sync.dma_start(w2_sb[:], w2_p[:])
            nc.sync.dma_start(wl_sb[:], wl_p[:])
            nc.sync.dma_start(b1_sb[:], b1_p[:])
            nc.sync.dma_start(b2_sb[:], b2_p[:])
            nc.sync.dma_start(bl_sb[:], bl_p[:])
            nc.gpsimd.memset(ones_sb[:], 1.0)
            make_identity(nc, ident_sb[:])
            nc.gpsimd.iota(iota_i[:], pattern=[[1, P]], base=0, channel_multiplier=0)
            nc.vector.tensor_copy(iota_f[:], iota_i[:])

            xT_sb = bigpool.tile([cfg.d_in, nloc], f32, tag="lhsT")
            gidxA_sb = bigpool.tile([P, ttotA * 8], i16)
            dstfA_sb = bigpool.tile([P, ttotA], f32)
            nrmfA_sb = bigpool.tile([P, ttotA], f32)
            gidxB_sb = bigpool.tile([P, ttotB * 8], i16)
            dstfB_sb = bigpool.tile([P, ttotB], f32)
            nrmfB_sb = bigpool.tile([P, ttotB], f32)
            pm_sb = bigpool.tile([P, nblk * g_], f32)
            nc.sync.dma_start(xT_sb[:], xT_p[:])
            nc.sync.dma_start(gidxA_sb[:], gidxA_p[:])
            nc.sync.dma_start(dstfA_sb[:], dstfA_p[:])
            nc.sync.dma_start(nrmfA_sb[:], nrmfA_p[:])
            nc.sync.dma_start(gidxB_sb[:], gidxB_p[:])
            nc.sync.dma_start(dstfB_sb[:], dstfB_p[:])
            nc.sync.dma_start(nrmfB_sb[:], nrmfB_p[:])
            nc.sync.dma_start(pm_sb[:], pm_p[:])

            hpre_sb = bigpool.tile([P, nloc], f32)

            def bsl(b, w=P):
                return slice(b * w, (b + 1) * w)

            def transform(lhsT_sb, w_sb, agin, table):
                for b in range(nblk):
                    ps = pspool.tile([P, dh], f32, tag="pst")
                    nc.tensor.matmul(ps[:], lhsT_sb[:, bsl(b)], w_sb[:],
                                     start=True, stop=True)
                    nc.vector.tensor_copy(hpre_sb[:, bsl(b)], ps[:])
                nc.sync.dma_start(
                    agin.ap().rearrange("(b p) f -> p b f", p=P),
                    hpre_sb[:].rearrange("p (b f) -> p b f", f=dh))
                nc.gpsimd.collective_compute(
                    "AllGather", mybir.AluOpType.bypass, replica_groups=rg,
                    ins=[agin[:]], outs=[table[:]])

            def aggregate(table, b_sb, hout_sb):
                streams = {
                    "A": dict(tstart=tstartA, ttot=ttotA, gidx=gidxA_sb,
                              dstf=dstfA_sb, nrmf=nrmfA_sb,
                              view=table[0:lo_split, :], cur=None, c0=-1),
                }
                if npad > lo_split and int(TB.sum()) > 0:
                    streams["B"] = dict(
                        tstart=tstartB, ttot=ttotB, gidx=gidxB_sb,
                        dstf=dstfB_sb, nrmf=nrmfB_sb,
                        view=table[lo_split:npad, :], cur=None, c0=-1)

                def fetch(s, t):
                    st = streams[s]
                    c0 = (t // chunk) * chunk
                    if st["c0"] != c0:
                        k = min(chunk, st["ttot"] - c0)
                        gt = gpool.tile([P, chunk, dh], f32, tag="g")
                        nc.gpsimd.dma_gather(
                            out_ap=gt[:, :k, :],
                            in_ap=st["view"],
                            idxs_ap=st["gidx"][:, c0 * 8:(c0 + k) * 8],
                            num_idxs=k * P,
                            num_idxs_reg=k * P,
                            elem_size=dh,
                        )
                        st["cur"], st["c0"] = gt, c0
                    return st["cur"][:, t - st["c0"], :]

                for b in range(nblk):
                    ps = pspool.tile([P, dh], f32, tag="psa")
                    first = True
                    for s in streams:
                        st = streams[s]
                        ts = st["tstart"]
                        for t in range(int(ts[b]), int(ts[b + 1])):
                            m_sb = mpool.tile([P, P], f32, tag="m")
                            nc.vector.tensor_scalar(
                                m_sb[:], iota_f[:],
                                scalar1=st["dstf"][:, t:t + 1],
                                scalar2=st["nrmf"][:, t:t + 1],
                                op0=ALU.is_equal, op1=ALU.mult)
                            rhs = fetch(s, t)
                            nc.tensor.matmul(ps[:], m_sb[:], rhs,
                                             start=first, stop=False)
                            first = False
                    nc.tensor.matmul(ps[:], ones_sb[:], b_sb[:],
                                     start=first, stop=True)
                    nc.scalar.activation(hout_sb[:, bsl(b)], ps[:], AF.Relu)

            # ---- layer 1 ----
            transform(xT_sb, w1_sb, agin1, table1)
            h1_sb = bigpool.tile([P, nloc], f32, tag="hout")
            aggregate(table1, b1_sb, h1_sb)

            # ---- layer 2 ----
            h1T_sb = bigpool.tile([P, nloc], f32, tag="lhsT")
            for b in range(nblk):
                ps = pspool.tile([P, P], f32, tag="pst")
                nc.tensor.transpose(ps[:], h1_sb[:, bsl(b)], ident_sb[:])
                nc.vector.tensor_copy(h1T_sb[:, bsl(b)], ps[:])
            transform(h1T_sb, w2_sb, agin2, table2)
            h2_sb = bigpool.tile([P, nloc], f32, tag="hout")
            aggregate(table2, b2_sb, h2_sb)

            # ---- pooling: sums^T[f, g] over blocks ----
            psp = pspool1.tile([P, g_], f32, tag="pool")
            for b in range(nblk):
                nc.tensor.matmul(psp[:], h2_sb[:, bsl(b)], pm_sb[:, bsl(b, g_)],
                                 start=(b == 0), stop=(b == nblk - 1))
            pool_sb = spool.tile([dh, g_], f32)
            nc.vector.tensor_copy(pool_sb[:], psp[:])
            nc.gpsimd.dma_start(arin[:], pool_sb[:])
            nc.gpsimd.collective_compute(
                "AllReduce", mybir.AluOpType.add, replica_groups=rg,
                ins=[arin[:]], outs=[arout[:]])
            mean_sb = spool.tile([dh, g_], f32)
            nc.sync.dma_start(mean_sb[:], arout[:])

            # ---- final linear + bias ----
            psg = pspool1.tile([g_, do], f32, tag="fin")
            nc.tensor.matmul(psg[:], mean_sb[:], wl_sb[:], start=True, stop=False)
            nc.tensor.matmul(psg[:], ones_sb[:, :g_], bl_sb[:], start=False, stop=True)
            g_sb = spool.tile([g_, do], f32)
            nc.vector.tensor_copy(g_sb[:], psg[:])

            # ---- L2 normalize rows ----
            sq_sb = spool.tile([g_, do], f32)
            s_sb = spool.tile([g_, 1], f32)
            nrm_sb = spool.tile([g_, 1], f32)
            inv_sb = spool.tile([g_, 1], f32)
            o_sb = spool.tile([g_, do], f32)
            nc.vector.tensor_mul(sq_sb[:], g_sb[:], g_sb[:])
            nc.vector.tensor_reduce(s_sb[:], sq_sb[:],
                                    axis=mybir.AxisListType.X, op=ALU.add)
            nc.scalar.sqrt(nrm_sb[:], s_sb[:])
            nc.vector.tensor_scalar_max(nrm_sb[:], nrm_sb[:], 1e-12)
            nc.vector.reciprocal(inv_sb[:], nrm_sb[:])
            nc.vector.tensor_scalar_mul(o_sb[:], g_sb[:], inv_sb[:, :1])
            nc.sync.dma_start(out_p[:], o_sb[:])

    nc.compile()
    return nc


_CACHE = {}
_LAST_EXEC_NS = None


def _run(cfg, x, W1, b1, W2, b2, Wl, bl, edge_index, batch, trace=False):
    from concourse.bass_utils import run_bass_kernel_spmd

    pre = preprocess(cfg, x, edge_index, batch)
    key = (cfg.n_nodes, cfg.nloc, pre["ttotA"], pre["ttotB"],
           tuple(pre["TA"].tolist()), tuple(pre["TB"].tolist()))
    if key not in _CACHE:
        _CACHE[key] = build(cfg, pre["TA"], pre["ttotA"], pre["TB"], pre["ttotB"])
    nc = _CACHE[key]

    in_maps = []
    for c in range(cfg.n_cores):
        m = {k + s: np.ascontiguousarray(pre[k + s][c])
             for s in ("A", "B") for k in ("gidx", "dstf", "nrmf")}
        m.update({
            "xT": np.ascontiguousarray(pre["xT"][c]),
            "pm": np.ascontiguousarray(pre["pm"][c]),
            "W1": np.asarray(W1, np.float32),
            "W2": np.asarray(W2, np.float32),
            "Wl": np.asarray(Wl, np.float32),
            "b1": np.asarray(b1, np.float32).reshape(1, -1),
            "b2": np.asarray(b2, np.float32).reshape(1, -1),
            "bl": np.asarray(bl, np.float32).reshape(1, -1),
        })
        in_maps.append(m)
    res = run_bass_kernel_spmd(nc, in_maps, list(range(cfg.n_cores)),
                               trace=trace)
    global _LAST_EXEC_NS
    _LAST_EXEC_NS = res.exec_time_ns
    return np.asarray(res.results[0]["out"], np.float32)


def kernel(x, W1, b1, W2, b2, Wl, bl, edge_index, batch):
    cfg = GCNConfig()
    return _run(cfg, x, W1, b1, W2, b2, Wl, bl, edge_index, batch)
